# revision 1
# baseline (speedup 1.0000x reference)
"""Trainium2 Bass kernel for DiscreteDeltaThetaGammaLayer.

Coupled Kuramoto-oscillator recurrence:
  phase0 = (x @ W_phase.T) mod 2pi ; amp0 = max(|x @ W_amp.T|, eps)
  32 steps of: intra-band Kuramoto coupling (phase), PAC amplitude modulation
  output: final amp  (4096, 352) f32

Strategy (8 NeuronCores, data-parallel over batch, 512 rows/core):
  - State held transposed [128 osc partitions x batch free] so the coupling
    matmul (s @ K.T) needs no per-step transposes. Oscillators permuted into
    chunks: c0 = delta(32)+theta(64)+pad(32), c1/c2 = gamma halves. Zero
    blocks of K.T are detected at runtime and their matmuls skipped.
  - Per-core batch split into three uneven independent streams (256/128/128)
    so the sequential recurrence of the streams pipelines across engines.
  - Phase kept wrapped in [-pi, pi] (ACT Sin LUT accurate there only);
    cos(phi) = sin(pi/2 - |phi|). Phase update fused into a custom DVE op
    WRAP_SUB: wrap((t - m2) + dt*omega) in one pass.
  - sin/cos written into one combined tile [cos | sin]; coupling results into
    one PSUM tensor [v | u]; so c*v and s*u are a single tensor_tensor pass.
  - Coupling + phase-projection matmuls in float32r (full PE rate at N>=256,
    ~1e-4 rel); the amp projection (the actual output) stays fp32.
  - Amp recurrence factored exactly: per-step band sums (PE matmuls into a
    batch-partition PSUM tile) are stashed and the host reconstructs
    f_k, prefix products P, running min m, amp = max(amp0*P, eps*P/m) --
    the exact closed form of the clamped recurrence.
"""

import math
import os
import sys

sys.path.insert(0, "/opt/trn_rl_repo")

import numpy as np

# ---- problem constants (module hyperparameters) ----
N_DELTA, N_THETA, N_GAMMA = 32, 64, 256
N_TOTAL = 352
N_DIMS = 1024
BATCH = 4096
N_STEPS = 32
DT = 0.01
PAC = 0.3
EPS = 1e-6
TWO_PI = 2.0 * math.pi
PI = math.pi

N_CORES = 8
BL = BATCH // N_CORES          # 512 batch rows per core
BHS = [256, 128, 128]          # uneven independent streams (latency hiding)
OFFS = [0, 256, 384]           # batch offset of each stream
NH = len(BHS)
P = 128
NCH = 3                        # oscillator chunks (3*128 = 384 >= 352)
CHUNK_REAL = [96, 128, 128]
KD = N_DIMS // P               # 8 contraction chunks for the projections

LAST_EXEC_NS = None
_COMPILED = {}
_WRAP_SUB = None


def _osc_perm():
    """orig oscillator index for each (chunk, partition); -1 for pads."""
    perm = -np.ones((NCH, P), dtype=np.int64)
    perm[0, :96] = np.arange(96)           # delta + theta
    perm[1, :] = 96 + np.arange(128)       # gamma 0:128
    perm[2, :] = 224 + np.arange(128)      # gamma 128:256
    return perm


def _get_wrap_sub():
    """Custom DVE op: out = wrap((in0 - in1) + s0) into [-s1, s1], period imm2."""
    global _WRAP_SUB
    if _WRAP_SUB is not None:
        return _WRAP_SUB
    from concourse.dve_spec import C0, C1, C2, Spec, Src0, Src1, lower
    from concourse.dve_uop import DveOpSpec
    import concourse.dve_ops as dvo

    def _ref(in0, in1, s0, s1, imm2):
        y = (in0 - in1) + s0
        return (y + imm2 * ((y < -s1).astype(np.float32)
                            - (y > s1).astype(np.float32))).astype(np.float32)

    _y = (Src0 - Src1) + C0
    spec = Spec(body=_y + C2 * ((_y < -C1) - (_y > C1)), reference=_ref)
    shas = {}
    for ver in ("v3", "v4"):
        tmp = DveOpSpec(name="WRAP_SUB_KERNEL", opcode=31,
                        uops=lower(spec, ver=ver), rd1_en=True)
        shas[ver] = tmp.sha(ver)
    op = dvo.DveOp("WRAP_SUB_KERNEL", spec, subdim=False, uops_sha=shas)
    dvo.OPS.append(op)
    dvo.CUSTOM_DVE_SPECS[op.name] = op.spec
    dvo._SUB_OPCODE_FOR_NAME[op.name] = dvo._CUSTOM_DVE_ROW_BASE + len(dvo.OPS) - 1
    _WRAP_SUB = op
    return op


def _build_program(nz_pairs, merge_g=False):
    import concourse.bass as bass
    import concourse.tile as tile
    from concourse import bacc, mybir

    wrap_sub = _get_wrap_sub()

    f32 = mybir.dt.float32
    f32r = mybir.dt.float32r
    AF = mybir.ActivationFunctionType
    ALU = mybir.AluOpType

    nc = bacc.Bacc("TRN2", target_bir_lowering=False, debug=False)

    # ---- DRAM I/O ----
    # xT and wpT are consumed by f32r matmuls (phase path); amp path reads the
    # same x bits through a bitcast-to-f32 view.
    xT = nc.dram_tensor("xT", [N_DIMS, BL], f32r, kind="ExternalInput").ap()
    xTf = nc.dram_tensor("xTf", [N_DIMS, BL], f32, kind="ExternalInput").ap()
    wpT = nc.dram_tensor("wpT", [N_DIMS, NCH * P], f32r, kind="ExternalInput").ap()
    waT = nc.dram_tensor("waT", [N_DIMS, NCH * P], f32, kind="ExternalInput").ap()
    kT = nc.dram_tensor("kT", [NCH * P, NCH * P], f32, kind="ExternalInput").ap()
    dtw = nc.dram_tensor("dtw", [P, NCH], f32, kind="ExternalInput").ap()
    wband = nc.dram_tensor("wband", [P, 2], f32, kind="ExternalInput").ap()

    amp0_out = nc.dram_tensor("amp0", [P, NCH * BL], f32, kind="ExternalOutput").ap()
    bs_out = nc.dram_tensor(
        "bsums", [P, N_STEPS * 16], f32, kind="ExternalOutput"
    ).ap()
    # bs_out = [stash0 (32*8) | stash1 (32*4) | stash2 (32*4)]; stash0 cols =
    # step*8 + q*4 + (Sd St Cd Ct), stash1/2 cols = step*4 + (Sd St Cd Ct).

    with tile.TileContext(nc) as tc:
        with (
            tc.tile_pool(name="state", bufs=1) as state_pool,
            tc.tile_pool(name="weights", bufs=1) as wpool,
            tc.tile_pool(name="work", bufs=3) as work,
            tc.tile_pool(name="psum", bufs=1, space="PSUM") as psum,
        ):
            # ---- persistent constants ----
            dtw_sb = wpool.tile([P, NCH], f32, tag="dtw")
            nc.gpsimd.dma_start(dtw_sb[:], dtw[:])
            pihalf = wpool.tile([P, 1], f32, tag="pihalf")
            nc.vector.memset(pihalf[:], PI / 2.0)
            wband_f = wpool.tile([P, 2], f32, tag="wband_f")
            nc.gpsimd.dma_start(wband_f[:], wband[:])
            wband_sb = wpool.tile([P, 2], f32r, tag="wband")
            nc.vector.tensor_copy(wband_sb[:], wband_f[:])

            kt_sb = {}
            for (jc, ic) in nz_pairs:
                tf = work.tile([P, P], f32, tag="ktld")
                nc.gpsimd.dma_start(tf[:], kT[jc * P:(jc + 1) * P, ic * P:(ic + 1) * P])
                t = wpool.tile([P, P], f32r, tag=f"kt_{jc}_{ic}")
                nc.vector.tensor_copy(t[:], tf[:])
                kt_sb[(jc, ic)] = t

            # ---- big input loads (split across DMA paths) ----
            # phase-path loads first: only they gate the recurrence; the amp
            # projection can lag and overlap the first steps.
            xk = []
            xkf = []
            wk_all = {}
            for k in range(KD):
                t = wpool.tile([P, BL], f32r, tag=f"x_{k}")
                eng = nc.gpsimd if k % 2 == 0 else nc.sync
                eng.dma_start(t[:], xT[k * P:(k + 1) * P, :])
                xk.append(t)
                t = wpool.tile([P, NCH * P], f32r, tag=f"w0_{k}")
                nc.sync.dma_start(t[:], wpT[k * P:(k + 1) * P, :])
                wk_all[(0, k)] = t
            for k in range(KD):
                t = wpool.tile([P, BL], f32, tag=f"xf_{k}")
                nc.gpsimd.dma_start(t[:], xTf[k * P:(k + 1) * P, :])
                xkf.append(t)
                t = wpool.tile([P, NCH * P], f32, tag=f"w1_{k}")
                nc.sync.dma_start(t[:], waT[k * P:(k + 1) * P, :])
                wk_all[(1, k)] = t

            # ---- per-stream state ----
            phi, stash, vu = [], [], []
            for h in range(NH):
                wh = NCH * BHS[h]
                nq = BHS[h] // P
                phi.append(state_pool.tile([P, wh], f32, tag=f"phi{h}",
                                           name=f"phi{h}"))
                stash.append(state_pool.tile([P, N_STEPS * 4 * nq], f32,
                                             tag=f"stash{h}", name=f"stash{h}"))
                vu.append(psum.tile([P, 2 * wh], f32, tag=f"vu{h}",
                                    name=f"vu{h}"))
            # one shared PSUM bank for every stream's band sums:
            # cols q*4 for stream0 q=0,1; col 8 for stream1; col 12 for stream2
            bs0 = psum.tile([P, 16], f32, tag="bs0", name="bs0")

            def bs_dst(h, q):
                if h == 0:
                    return bs0[:, q * 4:(q + 1) * 4]
                return bs0[:, (h + 1) * 4:(h + 2) * 4]

            # ---- initial projections (phase for every stream first) ----
            for proj, h in [(0, 0), (0, 1), (0, 2), (1, 0), (1, 1), (1, 2)]:
                bh = BHS[h]
                wh = NCH * bh
                if True:  # keep inner block indentation
                    dst = vu[h][:, proj * wh:(proj + 1) * wh]
                    for c in range(NCH):
                        acc = dst[:, c * bh:(c + 1) * bh]
                        for k in range(KD):
                            xsrc = xk[k] if proj == 0 else xkf[k]
                            rhs = xsrc[:, OFFS[h]:OFFS[h] + bh]
                            w = wk_all[(proj, k)][:, c * P:(c + 1) * P]
                            nc.tensor.matmul(
                                acc, w, rhs,
                                start=(k == 0),
                                stop=(k == KD - 1),
                            )
                    if proj == 0:
                        nc.vector.add_range_wrap(phi[h][:], dst, 0.0, PI, TWO_PI)
                    else:
                        ab = work.tile([P, wh], f32, tag=f"abs0_{h}",
                                       name=f"abs0_{h}")
                        nc.scalar.activation(ab[:], dst, AF.Abs)
                        for c in range(NCH):
                            nc.sync.dma_start(
                                amp0_out[:, c * BL + OFFS[h]:
                                         c * BL + OFFS[h] + bh],
                                ab[:, c * bh:(c + 1) * bh],
                            )

            # ---- the recurrence: NH independent streams ----
            for it in range(N_STEPS + 1):
                for h in range(NH):
                    bh = BHS[h]
                    wh = NCH * bh
                    nq = bh // P
                    ph = phi[h]
                    cs = work.tile([P, 2 * wh], f32r, tag=f"cs{h}", name=f"cs{h}")
                    sin = cs[:, wh:2 * wh]
                    cos = cs[:, 0:wh]
                    pabs = work.tile([P, wh], f32, tag=f"pabs{h}",
                                     name=f"pabs{h}")
                    # the final iteration only feeds the chunk-0 band sums
                    cw = bh if it == N_STEPS else wh
                    nc.scalar.activation(sin[:, 0:cw], ph[:, 0:cw], AF.Sin)
                    nc.scalar.activation(pabs[:, 0:cw], ph[:, 0:cw], AF.Abs)
                    nc.scalar.activation(cos[:, 0:cw], pabs[:, 0:cw], AF.Sin,
                                         bias=pihalf[:], scale=-1.0)

                    # band sums: cols (Sd St Cd Ct) per local slice q
                    if it > 0:
                        for q in range(nq):
                            dst = bs_dst(h, q)
                            nc.tensor.matmul(
                                dst[:, 0:2],
                                sin[:, q * P:(q + 1) * P],
                                wband_sb[:],
                                start=True, stop=True,
                            )
                            nc.tensor.matmul(
                                dst[:, 2:4],
                                cos[:, q * P:(q + 1) * P],
                                wband_sb[:],
                                start=True, stop=True,
                            )
                            if h > 0:
                                nc.scalar.copy(
                                    stash[h][:, (it - 1) * 4:it * 4],
                                    dst[:, 0:4],
                                )
                        if h == 0:
                            nc.scalar.copy(
                                stash[0][:, (it - 1) * 8:it * 8], bs0[:, 0:8]
                            )

                    if it == N_STEPS:
                        continue

                    # coupling: [v | u] = (dt*K) [sin | cos]  (f32r matmuls)
                    for ic in range(NCH):
                        jcs = [jc for (jc, i2) in nz_pairs if i2 == ic]
                        for half, src in ((0, sin), (1, cos)):
                            dst = vu[h][:, half * wh + ic * bh:
                                        half * wh + (ic + 1) * bh]
                            for n, jc in enumerate(jcs):
                                nc.tensor.matmul(
                                    dst,
                                    kt_sb[(jc, ic)][:],
                                    src[:, jc * bh:(jc + 1) * bh],
                                    start=(n == 0), stop=(n == len(jcs) - 1),
                                )

                    # mm = [cos|sin] * [v|u] -> [m1 | m2]  (single TT pass)
                    mm = work.tile([P, 2 * wh], f32, tag=f"mm{h}", name=f"mm{h}")
                    nc.vector.tensor_tensor(
                        mm[:], cs[:].bitcast(f32), vu[h][:, 0:2 * wh], ALU.mult
                    )
                    # t = phi + m1 ; phi = wrap((t - m2) + dt*omega)
                    t = work.tile([P, wh], f32, tag=f"t{h}", name=f"t{h}")
                    nc.vector.tensor_tensor(t[:], ph[:], mm[:, 0:wh], ALU.add)
                    # chunks 1,2 share a dtw vector when omega is band-constant
                    spans = [(0, 1), (1, 3)] if merge_g else [(0, 1), (1, 2), (2, 3)]
                    for c0, c1 in spans:
                        nc.vector._custom_dve(
                            wrap_sub,
                            out=ph[:, c0 * bh:c1 * bh],
                            in0=t[:, c0 * bh:c1 * bh],
                            in1=mm[:, wh + c0 * bh:wh + c1 * bh],
                            s0=dtw_sb[:, c0:c0 + 1],
                            s1=PI,
                            imm2=TWO_PI,
                        )

            # ---- outputs ----
            # bsums: contiguous per-stream stashes; host decodes layouts
            off = 0
            for h in range(NH):
                n = N_STEPS * 4 * (BHS[h] // P)
                nc.sync.dma_start(bs_out[:, off:off + n], stash[h][:])
                off += n

    nc.compile()
    return nc


def kernel(x, W_phase, W_amp, omega, K):
    from concourse.bass_utils import run_bass_kernel_spmd

    x = np.asarray(x, dtype=np.float32)
    W_phase = np.asarray(W_phase, dtype=np.float32)
    W_amp = np.asarray(W_amp, dtype=np.float32)
    omega = np.asarray(omega, dtype=np.float32)
    K = np.asarray(K, dtype=np.float32)

    perm = _osc_perm()

    # ---- host-side packing ----
    wpT = np.zeros((N_DIMS, NCH * P), dtype=np.float32)
    waT = np.zeros((N_DIMS, NCH * P), dtype=np.float32)
    dtw = np.zeros((P, NCH), dtype=np.float32)
    for c in range(NCH):
        n = CHUNK_REAL[c]
        idx = perm[c, :n]
        wpT[:, c * P:c * P + n] = W_phase[idx].T
        waT[:, c * P:c * P + n] = W_amp[idx].T
        w = DT * omega[idx].astype(np.float64)
        dtw[:n, c] = (np.mod(w + PI, TWO_PI) - PI).astype(np.float32)

    kT = np.zeros((NCH * P, NCH * P), dtype=np.float32)
    for jc in range(NCH):
        nj = CHUNK_REAL[jc]
        jdx = perm[jc, :nj]
        for ic in range(NCH):
            ni = CHUNK_REAL[ic]
            idx = perm[ic, :ni]
            kT[jc * P:jc * P + nj, ic * P:ic * P + ni] = DT * K[np.ix_(idx, jdx)].T

    nz = [
        (jc, ic)
        for jc in range(NCH)
        for ic in range(NCH)
        if np.any(kT[jc * P:(jc + 1) * P, ic * P:(ic + 1) * P] != 0.0)
    ]
    # every output chunk needs at least one matmul so its PSUM slice is
    # written (zero block is fine)
    for ic in range(NCH):
        if not any(i2 == ic for (_, i2) in nz):
            nz.append((ic, ic))
    nz_pairs = tuple(sorted(nz))

    wband = np.zeros((P, 2), dtype=np.float32)
    wband[:N_DELTA, 0] = 1.0
    wband[N_DELTA:N_DELTA + N_THETA, 1] = 1.0

    merge_g = bool(np.array_equal(dtw[:, 1], dtw[:, 2]))
    key = (nz_pairs, merge_g)
    if key not in _COMPILED:
        _COMPILED[key] = _build_program(nz_pairs, merge_g)
    nc = _COMPILED[key]

    in_maps = []
    for i in range(N_CORES):
        xs = x[i * BL:(i + 1) * BL]
        xst = np.ascontiguousarray(xs.T)
        in_maps.append({
            "xT": xst, "xTf": xst,
            "wpT": wpT, "waT": waT, "kT": kT, "dtw": dtw, "wband": wband,
        })

    res = run_bass_kernel_spmd(nc, in_maps, core_ids=list(range(N_CORES)))

    # ---- host-side unshard + exact amp reconstruction ----
    band_of = np.zeros(N_TOTAL, dtype=np.int64)
    band_of[N_DELTA:N_DELTA + N_THETA] = 1
    band_of[N_DELTA + N_THETA:] = 2

    out = np.empty((BATCH, N_TOTAL), dtype=np.float32)
    for i in range(N_CORES):
        r = res.results[i]
        amp0v = np.maximum(np.abs(r["amp0"].astype(np.float64)), EPS)
        bsv = r["bsums"].astype(np.float64)
        bss = np.empty((P, N_STEPS, 4, 4))
        bss[:, :, 0:2, :] = bsv[:, 0:N_STEPS * 8].reshape(P, N_STEPS, 2, 4)
        bss[:, :, 2, :] = bsv[:, N_STEPS * 8:N_STEPS * 12].reshape(P, N_STEPS, 4)
        bss[:, :, 3, :] = bsv[:, N_STEPS * 12:N_STEPS * 16].reshape(P, N_STEPS, 4)
        S = bss[:, :, :, 0:2]                       # [p, k, q, band]
        C = bss[:, :, :, 2:4]
        cosm = C / np.sqrt(S * S + C * C)
        f = 1.0 + DT * PAC * cosm                   # [p, k, q, band]
        Pk = np.cumprod(f, axis=1)
        m = np.minimum.accumulate(Pk, axis=1)
        Pn = Pk[:, -1]                              # [p, q, band]
        mn = m[:, -1]
        Pfac = np.ones((BL, 3))
        Efac = np.ones((BL, 3))
        for q in range(4):
            sl = slice(q * P, (q + 1) * P)
            Pfac[sl, 1] = Pn[:, q, 0]
            Pfac[sl, 2] = Pn[:, q, 1]
            Efac[sl, 1] = Pn[:, q, 0] / mn[:, q, 0]
            Efac[sl, 2] = Pn[:, q, 1] / mn[:, q, 1]
        a0 = np.empty((BL, N_TOTAL))
        for c in range(NCH):
            n = CHUNK_REAL[c]
            idx = perm[c, :n]
            a0[:, idx] = amp0v[:n, c * BL:(c + 1) * BL].T
        amp = np.maximum(a0 * Pfac[:, band_of], EPS * Efac[:, band_of])
        out[i * BL:(i + 1) * BL] = amp.astype(np.float32)
    return out



# revision 8
# speedup vs baseline: 1.1709x; 1.1709x over previous
"""Trainium2 Bass kernel for DiscreteDeltaThetaGammaLayer.

Coupled Kuramoto-oscillator recurrence:
  phase0 = (x @ W_phase.T) mod 2pi ; amp0 = max(|x @ W_amp.T|, eps)
  32 steps of: intra-band Kuramoto coupling (phase), PAC amplitude modulation
  output: final amp  (4096, 352) f32

Strategy (8 NeuronCores, data-parallel over batch, 512 rows/core):
  - State transposed [128 osc partitions x batch free]; oscillators permuted
    into chunks: c0 = delta(32)+theta(64)+pad(32), c1/c2 = gamma halves.
  - Rotating frame per band: phi~ = phi - k*dt*omega_band. The per-step
    dt*omega add AND the wrap disappear (coupling drift <= 0.02 rad/step,
    32 steps => |phi~| <= pi+0.65 where the Sin LUT still has ~1e-3 abs err).
    Host de-rotates the stashed band sums exactly in f64.
  - bf16 state + coupling matmuls (full PE rate at any width), f32r amp path.
  - Band sums (Sd St Cd Ct) fused into the chunk-0 coupling matmul: the
    K-block's 32 pad lhsT columns carry delta/theta indicator columns, so
    PSUM partitions 96:98 of vu hold the band sums for free.
  - Per step per stream: 2 ACT sin (cos via sin(pi/2-|phi|)), 1 ACT abs,
    10 PE matmuls, DVE: mm=cs*vu, d=mm1-mm2, phi+=d (bf16 TT at 2x),
    Pool: stash copy. Host reconstructs amp exactly (clamped-recurrence
    closed form) from stashed band sums.
  - Fallback (general omega / huge coupling): no rotating frame, per-step
    custom-DVE wrap with dt*omega folded in (s0).
"""

import math
import sys

sys.path.insert(0, "/opt/trn_rl_repo")

import numpy as np

# ---- problem constants (module hyperparameters) ----
N_DELTA, N_THETA, N_GAMMA = 32, 64, 256
N_TOTAL = 352
N_DIMS = 1024
BATCH = 4096
N_STEPS = 32
DT = 0.01
PAC = 0.3
EPS = 1e-6
TWO_PI = 2.0 * math.pi
PI = math.pi

N_CORES = 8
BL = BATCH // N_CORES          # 512 batch rows per core
BHS = [256, 256]               # independent streams (latency hiding)
OFFS = [0, 256]
NH = len(BHS)
P = 128
NCH = 3                        # oscillator chunks (3*128 = 384 >= 352)
CHUNK_REAL = [96, 128, 128]
KD = N_DIMS // P               # 8 contraction chunks for the projections

LAST_EXEC_NS = None
_COMPILED = {}
_WRAP_SUB = None

# drift budget: |phi~| may reach pi + DRIFT_MAX with Sin LUT err ~1.2e-3
DRIFT_MAX = 0.66


def _osc_perm():
    """orig oscillator index for each (chunk, partition); -1 for pads."""
    perm = -np.ones((NCH, P), dtype=np.int64)
    perm[0, :96] = np.arange(96)           # delta + theta
    perm[1, :] = 96 + np.arange(128)       # gamma 0:128
    perm[2, :] = 224 + np.arange(128)      # gamma 128:256
    return perm


def _get_wrap_sub():
    """Custom DVE op: out = wrap((in0 - in1) + s0) into [-s1, s1], period imm2."""
    global _WRAP_SUB
    if _WRAP_SUB is not None:
        return _WRAP_SUB
    from concourse.dve_spec import C0, C1, C2, Spec, Src0, Src1, lower
    from concourse.dve_uop import DveOpSpec
    import concourse.dve_ops as dvo

    def _ref(in0, in1, s0, s1, imm2):
        y = (in0 - in1) + s0
        return (y + imm2 * ((y < -s1).astype(np.float32)
                            - (y > s1).astype(np.float32))).astype(np.float32)

    _y = (Src0 - Src1) + C0
    spec = Spec(body=_y + C2 * ((_y < -C1) - (_y > C1)), reference=_ref)
    shas = {}
    for ver in ("v3", "v4"):
        tmp = DveOpSpec(name="WRAP_SUB_KERNEL", opcode=31,
                        uops=lower(spec, ver=ver), rd1_en=True)
        shas[ver] = tmp.sha(ver)
    op = dvo.DveOp("WRAP_SUB_KERNEL", spec, subdim=False, uops_sha=shas)
    dvo.OPS.append(op)
    dvo.CUSTOM_DVE_SPECS[op.name] = op.spec
    dvo._SUB_OPCODE_FOR_NAME[op.name] = dvo._CUSTOM_DVE_ROW_BASE + len(dvo.OPS) - 1
    _WRAP_SUB = op
    return op


def _build_program(nz_pairs, fast_rot, has_res):
    """fast_rot: rotating frame, no wrap. has_res: per-osc omega residual."""
    import concourse.bass as bass
    import concourse.tile as tile
    from concourse import bacc, mybir

    f32 = mybir.dt.float32
    f32r = mybir.dt.float32r
    bf16 = mybir.dt.bfloat16
    AF = mybir.ActivationFunctionType
    ALU = mybir.AluOpType

    wrap_sub = _get_wrap_sub() if not fast_rot else None

    nc = bacc.Bacc("TRN2", target_bir_lowering=False, debug=False)

    # ---- DRAM I/O ----
    xbT = nc.dram_tensor("xbT", [N_DIMS, BL], bf16, kind="ExternalInput").ap()
    wpT = nc.dram_tensor("wpT", [N_DIMS, NCH * P], bf16, kind="ExternalInput").ap()
    xfT = nc.dram_tensor("xfT", [N_DIMS, BL], f32r, kind="ExternalInput").ap()
    waT = nc.dram_tensor("waT", [N_DIMS, NCH * P], f32r, kind="ExternalInput").ap()
    ktT = nc.dram_tensor("ktT", [NCH * P, NCH * P], bf16, kind="ExternalInput").ap()
    # per-(partition,chunk) scalars: residual r (fast path) or dt*omega (fallback)
    dtw = nc.dram_tensor("dtw", [P, NCH], f32, kind="ExternalInput").ap()
    # tap-partition phase init rows: [0, 0, pi/2, pi/2] x bh
    padphi = nc.dram_tensor("padphi", [4, max(BHS)], bf16,
                            kind="ExternalInput").ap()

    amp0_out = nc.dram_tensor("amp0", [P, NCH * BL], f32, kind="ExternalOutput").ap()
    # stash: rows (Sd, St, -Cd, -Ct); per stream block of N_STEPS*bh cols
    bs_out = nc.dram_tensor("bsums", [4, N_STEPS * BL], bf16,
                            kind="ExternalOutput").ap()

    with tile.TileContext(nc) as tc:
        with (
            tc.tile_pool(name="state", bufs=1) as state_pool,
            tc.tile_pool(name="weights", bufs=1) as wpool,
            tc.tile_pool(name="work", bufs=2) as work,
            tc.tile_pool(name="psum", bufs=1, space="PSUM") as psum,
        ):
            # ---- persistent constants ----
            dtw_sb = None
            if (not fast_rot) or has_res:
                dtw_sb = wpool.tile([P, NCH], f32, tag="dtw", name="dtw_sb")
                nc.gpsimd.dma_start(dtw_sb[:], dtw[:])
            pihalf = wpool.tile([P, 1], f32, tag="pihalf", name="pihalf")
            nc.vector.memset(pihalf[:], PI / 2.0)

            kt_sb = {}
            for (jc, ic) in nz_pairs:
                t = wpool.tile([P, P], bf16, tag=f"kt_{jc}_{ic}",
                               name=f"kt_{jc}_{ic}")
                nc.gpsimd.dma_start(
                    t[:], ktT[jc * P:(jc + 1) * P, ic * P:(ic + 1) * P])
                kt_sb[(jc, ic)] = t

            # ---- big input loads (phase path first: it gates the recurrence)
            xk, wk = [], []
            for k in range(KD):
                t = wpool.tile([P, BL], bf16, tag=f"x_{k}", name=f"x_{k}")
                eng = nc.gpsimd if k % 2 == 0 else nc.sync
                eng.dma_start(t[:], xbT[k * P:(k + 1) * P, :])
                xk.append(t)
                t = wpool.tile([P, NCH * P], bf16, tag=f"w0_{k}", name=f"w0_{k}")
                nc.sync.dma_start(t[:], wpT[k * P:(k + 1) * P, :])
                wk.append(t)
            xfk, wak = [], []
            for k in range(KD):
                t = wpool.tile([P, BL], f32r, tag=f"xf_{k}", name=f"xf_{k}")
                nc.gpsimd.dma_start(t[:], xfT[k * P:(k + 1) * P, :])
                xfk.append(t)
                t = wpool.tile([P, NCH * P], f32r, tag=f"w1_{k}", name=f"w1_{k}")
                nc.sync.dma_start(t[:], waT[k * P:(k + 1) * P, :])
                wak.append(t)

            boff = [N_STEPS * sum(BHS[:h]) for h in range(NH)]
            # ---- per-stream state ----
            phi, cs, mmt, dts, vu = [], [], [], [], []
            for h in range(NH):
                bh = BHS[h]
                wh = NCH * bh
                phi.append(state_pool.tile([P, wh], bf16, tag=f"phi{h}",
                                           name=f"phi{h}"))
                cs.append(state_pool.tile([P, 2 * wh], bf16, tag=f"cs{h}",
                                          name=f"cs{h}"))
                mmt.append(state_pool.tile([P, 2 * wh], bf16, tag=f"mm{h}",
                                           name=f"mm{h}"))
                dts.append(state_pool.tile([P, wh], bf16, tag=f"d{h}",
                                           name=f"d{h}"))
                vu.append(psum.tile([P, 2 * wh], f32, tag=f"vu{h}",
                                    name=f"vu{h}"))
            amp_acc = psum.tile([P, NCH * max(BHS)], f32, tag="ampacc",
                                name="amp_acc")
            pabs = [work.tile([P, NCH * BHS[h]], bf16, tag=f"pabs{h}",
                              name=f"pabs{h}") for h in range(NH)]

            # ---- phase projections -> phi (per stream) ----
            for h in range(NH):
                bh = BHS[h]
                wh = NCH * bh
                for c in range(NCH):
                    acc = vu[h][:, c * bh:(c + 1) * bh]
                    for k in range(KD):
                        nc.tensor.matmul(
                            acc, wk[k][:, c * P:(c + 1) * P],
                            xk[k][:, OFFS[h]:OFFS[h] + bh],
                            start=(k == 0), stop=(k == KD - 1),
                        )
                nc.vector.add_range_wrap(phi[h][:], vu[h][:, 0:wh],
                                         0.0, PI, TWO_PI)
                # pad partitions of chunk 0 carry band-sum taps:
                # 96,97 keep phi=0 (cos=1,sin=0); 98,99 get pi/2 (cos=0,sin=1)
                nc.gpsimd.dma_start(phi[h][96:100, 0:bh], padphi[:, 0:bh])

            # ---- the recurrence ----
            for it in range(N_STEPS + 1):
                for h in range(NH):
                    bh = BHS[h]
                    wh = NCH * bh
                    ph = phi[h]
                    sin = cs[h][:, wh:2 * wh]
                    cos = cs[h][:, 0:wh]
                    # last iteration only feeds the chunk-0 band sums
                    cw = bh if it == N_STEPS else wh
                    nc.scalar.activation(sin[:, 0:cw], ph[:, 0:cw], AF.Sin)
                    nc.scalar.activation(pabs[h][:, 0:cw], ph[:, 0:cw], AF.Abs)
                    nc.scalar.activation(cos[:, 0:cw], pabs[h][:, 0:cw], AF.Sin,
                                         bias=pihalf[:], scale=-1.0)

                    # coupling: [v | u] = (dt*K) [sin | cos]; chunk-0 block
                    # also emits band sums on partitions 96:98
                    for ic in range(NCH):
                        if it == N_STEPS and ic > 0:
                            continue
                        jcs = [jc for (jc, i2) in nz_pairs if i2 == ic]
                        for half, srcoff in ((0, wh), (1, 0)):
                            dst = vu[h][:, half * wh + ic * bh:
                                        half * wh + (ic + 1) * bh]
                            for n, jc in enumerate(jcs):
                                src = cs[h][:, srcoff + jc * bh:
                                            srcoff + (jc + 1) * bh]
                                nc.tensor.matmul(
                                    dst, kt_sb[(jc, ic)][:], src,
                                    start=(n == 0), stop=(n == len(jcs) - 1),
                                )

                    # stash band sums of post-update phase (it >= 1):
                    # vu partitions 96:98, chunk0 of each half -> stash cols
                    if it == N_STEPS:
                        # band sums only: mm and d on tap partitions, chunk 0
                        for half in (0, 1):
                            nc.vector.tensor_tensor(
                                mmt[h][96:100, half * wh:half * wh + bh],
                                cs[h][96:100, half * wh:half * wh + bh],
                                vu[h][96:100, half * wh:half * wh + bh],
                                ALU.mult)
                        a, b = (0, wh) if fast_rot else (wh, 0)
                        nc.vector.tensor_tensor(
                            dts[h][96:100, 0:bh],
                            mmt[h][96:100, a:a + bh],
                            mmt[h][96:100, b:b + bh], ALU.subtract)
                        so = boff[h] + (it - 1) * bh
                        nc.gpsimd.dma_start(bs_out[:, so:so + bh],
                                            dts[h][96:100, 0:bh])
                        continue

                    # mm = [cos|sin] * [v|u]
                    nc.vector.tensor_tensor(mmt[h][:], cs[h][:], vu[h][:],
                                            ALU.mult)
                    # fast path: d = c*v - s*u (= coup); fallback: d = -coup
                    # since WRAP_SUB computes wrap((phi - d) + s0).
                    # tap partitions 96:100 of chunk 0 hold (Sd, St, -Cd, -Ct)
                    # (negated in fallback mode).
                    if fast_rot:
                        nc.vector.tensor_tensor(
                            dts[h][:], mmt[h][:, 0:wh],
                            mmt[h][:, wh:2 * wh], ALU.subtract)
                    else:
                        nc.vector.tensor_tensor(
                            dts[h][:], mmt[h][:, wh:2 * wh],
                            mmt[h][:, 0:wh], ALU.subtract)
                    if it > 0:
                        so = boff[h] + (it - 1) * bh
                        nc.gpsimd.dma_start(bs_out[:, so:so + bh],
                                            dts[h][96:100, 0:bh])
                    if fast_rot:
                        if has_res:
                            for c in range(NCH):
                                pe = 96 if c == 0 else P
                                nc.vector.scalar_tensor_tensor(
                                    ph[0:pe, c * bh:(c + 1) * bh],
                                    dts[h][0:pe, c * bh:(c + 1) * bh],
                                    dtw_sb[0:pe, c:c + 1],
                                    ph[0:pe, c * bh:(c + 1) * bh],
                                    ALU.add, ALU.add)
                        else:
                            nc.vector.tensor_tensor(
                                ph[0:96, 0:bh], ph[0:96, 0:bh],
                                dts[h][0:96, 0:bh], ALU.add)
                            nc.vector.tensor_tensor(
                                ph[:, bh:wh], ph[:, bh:wh],
                                dts[h][:, bh:wh], ALU.add)
                    else:
                        for c in range(NCH):
                            pe = 96 if c == 0 else P
                            nc.vector._custom_dve(
                                wrap_sub,
                                out=ph[0:pe, c * bh:(c + 1) * bh],
                                in0=ph[0:pe, c * bh:(c + 1) * bh],
                                in1=dts[h][0:pe, c * bh:(c + 1) * bh],
                                s0=dtw_sb[0:pe, c:c + 1],
                                s1=PI,
                                imm2=TWO_PI,
                            )

            # ---- amp projections (PE lags behind recurrence start) ----
            for h in range(NH):
                bh = BHS[h]
                wh = NCH * bh
                for c in range(NCH):
                    acc = amp_acc[:, c * bh:(c + 1) * bh]
                    for k in range(KD):
                        nc.tensor.matmul(
                            acc, wak[k][:, c * P:(c + 1) * P],
                            xfk[k][:, OFFS[h]:OFFS[h] + bh],
                            start=(k == 0), stop=(k == KD - 1),
                        )
                ab = work.tile([P, wh], f32, tag=f"abs0_{h}", name=f"abs0_{h}")
                nc.scalar.activation(ab[:], amp_acc[:, 0:wh], AF.Abs)
                for c in range(NCH):
                    nc.sync.dma_start(
                        amp0_out[:, c * BL + OFFS[h]:c * BL + OFFS[h] + bh],
                        ab[:, c * bh:(c + 1) * bh],
                    )


    nc.compile()
    return nc


def kernel(x, W_phase, W_amp, omega, K):
    import ml_dtypes
    from concourse.bass_utils import run_bass_kernel_spmd

    x = np.asarray(x, dtype=np.float32)
    W_phase = np.asarray(W_phase, dtype=np.float32)
    W_amp = np.asarray(W_amp, dtype=np.float32)
    omega = np.asarray(omega, dtype=np.float32)
    K = np.asarray(K, dtype=np.float32)

    perm = _osc_perm()
    band_of = np.zeros(N_TOTAL, dtype=np.int64)
    band_of[N_DELTA:N_DELTA + N_THETA] = 1
    band_of[N_DELTA + N_THETA:] = 2

    # ---- rotating-frame feasibility ----
    dtww = DT * omega.astype(np.float64)
    A_band = np.array([dtww[band_of == b].mean() for b in range(3)])
    res = dtww - A_band[band_of]                      # per-osc residual
    # coupling drift bound
    row_l1 = DT * np.abs(K.astype(np.float64)).sum(axis=1)
    drift = N_STEPS * (np.abs(res) + row_l1).max()
    # coupled pairs must share a frame rate
    ii, jj = np.nonzero(K)
    frames_ok = np.allclose(A_band[band_of[ii]], A_band[band_of[jj]],
                            rtol=0, atol=1e-12) if len(ii) else True
    fast_rot = bool(frames_ok and drift <= DRIFT_MAX)
    has_res = bool(fast_rot and np.abs(res).max() > 1e-12)

    # ---- host-side packing ----
    wpT = np.zeros((N_DIMS, NCH * P), dtype=ml_dtypes.bfloat16)
    waT = np.zeros((N_DIMS, NCH * P), dtype=np.float32)
    dtw = np.zeros((P, NCH), dtype=np.float32)
    for c in range(NCH):
        n = CHUNK_REAL[c]
        idx = perm[c, :n]
        wpT[:, c * P:c * P + n] = W_phase[idx].T.astype(ml_dtypes.bfloat16)
        waT[:, c * P:c * P + n] = W_amp[idx].T
        if fast_rot:
            dtw[:n, c] = res[idx].astype(np.float32)
        else:
            w = dtww[idx]
            dtw[:n, c] = (np.mod(w + PI, TWO_PI) - PI).astype(np.float32)

    kT = np.zeros((NCH * P, NCH * P), dtype=np.float32)
    for jc in range(NCH):
        nj = CHUNK_REAL[jc]
        jdx = perm[jc, :nj]
        for ic in range(NCH):
            ni = CHUNK_REAL[ic]
            idx = perm[ic, :ni]
            kT[jc * P:jc * P + nj, ic * P:ic * P + ni] = \
                DT * K[np.ix_(idx, jdx)].T

    nz = [
        (jc, ic)
        for jc in range(NCH)
        for ic in range(NCH)
        if np.any(kT[jc * P:(jc + 1) * P, ic * P:(ic + 1) * P] != 0.0)
    ]
    if (0, 0) not in nz:
        nz.append((0, 0))     # carries the band-sum indicator columns
    for ic in range(1, NCH):
        if not any(i2 == ic for (_, i2) in nz):
            nz.append((ic, ic))
    nz_pairs = tuple(sorted(nz))

    # fuse delta/theta indicator columns into the (0,0) block pads:
    # cols 96,97 tap the sin half (phi_pad=0), cols 98,99 the cos half
    # (phi_pad=pi/2)
    for cc in (96, 98):
        kT[0:N_DELTA, cc] = 1.0
        kT[N_DELTA:96, cc + 1] = 1.0
    ktT = kT.astype(ml_dtypes.bfloat16)

    key = (nz_pairs, fast_rot, has_res)
    if key not in _COMPILED:
        _COMPILED[key] = _build_program(nz_pairs, fast_rot, has_res)
    nc = _COMPILED[key]

    in_maps = []
    for i in range(N_CORES):
        xs = x[i * BL:(i + 1) * BL]
        xst = np.ascontiguousarray(xs.T)
        padphi = np.zeros((4, max(BHS)), dtype=ml_dtypes.bfloat16)
        padphi[2:4, :] = np.float32(PI / 2.0)
        in_maps.append({
            "xbT": xst.astype(ml_dtypes.bfloat16), "xfT": xst,
            "wpT": wpT, "waT": waT, "ktT": ktT, "dtw": dtw,
            "padphi": padphi,
        })

    res_run = run_bass_kernel_spmd(nc, in_maps, core_ids=list(range(N_CORES)))

    # ---- host-side unshard + exact amp reconstruction (f64) ----
    out = np.empty((BATCH, N_TOTAL), dtype=np.float32)
    ks = np.arange(1, N_STEPS + 1, dtype=np.float64)   # stash it index
    # de-rotation phases per band (delta for theta-mod, theta for gamma-mod)
    if fast_rot:
        rotd = ks * A_band[0]
        rott = ks * A_band[1]
    else:
        rotd = np.zeros(N_STEPS)
        rott = np.zeros(N_STEPS)

    for i in range(N_CORES):
        r = res_run.results[i]
        amp0v = np.maximum(np.abs(r["amp0"].astype(np.float64)), EPS)
        bsv = r["bsums"].astype(np.float64)      # [4, N_STEPS*BL]
        if not fast_rot:
            bsv = -bsv                           # fallback d = -coup sign
        # per-stream decode -> f-factors [BL, N_STEPS, {theta, gamma}]
        f = np.empty((BL, N_STEPS, 2))
        off = 0
        for h in range(NH):
            bh = BHS[h]
            blk = bsv[:, off:off + N_STEPS * bh].reshape(4, N_STEPS, bh)
            S = blk[0:2]                          # [2(d,t), k, j] sin sums
            C = -blk[2:4]
            R = np.sqrt(S * S + C * C)
            R = np.maximum(R, 1e-30)
            # true cos(mean phase) = (C cos(kA) - S sin(kA)) / R
            cd = (C[0] * np.cos(rotd)[:, None]
                  - S[0] * np.sin(rotd)[:, None]) / R[0]
            ct = (C[1] * np.cos(rott)[:, None]
                  - S[1] * np.sin(rott)[:, None]) / R[1]
            sl = slice(OFFS[h], OFFS[h] + bh)
            f[sl, :, 0] = 1.0 + DT * PAC * cd.T   # theta-band factor
            f[sl, :, 1] = 1.0 + DT * PAC * ct.T   # gamma-band factor
            off += N_STEPS * bh
        Pk = np.cumprod(f, axis=1)                # [BL, k, 2]
        m = np.minimum.accumulate(Pk, axis=1)
        Pn = Pk[:, -1]                            # [BL, 2]
        mn = m[:, -1]
        Pfac = np.ones((BL, 3))
        Efac = np.ones((BL, 3))
        Pfac[:, 1:] = Pn
        Efac[:, 1:] = Pn / mn
        a0 = np.empty((BL, N_TOTAL))
        for c in range(NCH):
            n = CHUNK_REAL[c]
            idx = perm[c, :n]
            a0[:, idx] = amp0v[:n, c * BL:(c + 1) * BL].T
        amp = np.maximum(a0 * Pfac[:, band_of], EPS * Efac[:, band_of])
        out[i * BL:(i + 1) * BL] = amp.astype(np.float32)
    return out
